# revision 1
# baseline (speedup 1.0000x reference)
"""Trainium2 Bass kernel for nn_Loss_46883863003176.

loss = sum((predictions - targets)**2) / (2d+1) / batch_size
with predictions/targets of shape (4096, 2047, 2) float32.

Strategy (data-parallel over 8 NeuronCores):
  - Each core gets a contiguous batch shard of 512 rows, viewed flat as
    [128 partitions, 16376] f32 per tensor (33.5 MB HBM traffic per core).
  - Per tile of [128, F]: HWDGE DMA loads of pred/targ, DVE tensor_sub
    computes diff, ACT Square activation with accum_out produces the
    per-partition running sum of squares. Memory-bound; DVE and ACT each
    stay well under the ~93 us/core HBM roofline.
  - Each core writes a [128, NT] partial-sum tensor; host sums the 8
    partials in float64 and divides by (2d+1)*batch_size.
"""

import sys

if "/opt/trn_rl_repo" not in sys.path:
    sys.path.insert(0, "/opt/trn_rl_repo")

import numpy as np

B = 4096          # batch
S = 2047          # 2*d+1
C = 2             # coords
N_CORES = 8
ROWS = B // N_CORES          # 512 batch rows per core
PER_CORE = ROWS * S * C      # 2,096,128 elements
P = 128                      # SBUF partitions
FREE = PER_CORE // P         # 16376 elements per partition
# Tapered tile sizes (elements per partition). Large tiles amortize DMA
# issue cost mid-stream; small trailing tiles shrink the compute tail that
# runs after the last DMA completes. Must sum to FREE.
TILE_SIZES = [8188, 4094, 2047, 2047]
assert sum(TILE_SIZES) == FREE
NT = len(TILE_SIZES)

_CACHE = {}


def _build():
    import concourse.tile as tile
    from concourse import bacc, mybir

    nc = bacc.Bacc(
        "TRN2", debug=False, target_bir_lowering=False, num_devices=N_CORES
    )
    f32 = mybir.dt.float32
    p_ap = nc.dram_tensor("p", [P, FREE], f32, kind="ExternalInput").ap()
    t_ap = nc.dram_tensor("t", [P, FREE], f32, kind="ExternalInput").ap()
    acc_ap = nc.dram_tensor("acc", [P, NT], f32, kind="ExternalOutput").ap()

    with tile.TileContext(nc) as tc:
        with (
            tc.tile_pool(name="io", bufs=3) as io_pool,
            tc.tile_pool(name="work", bufs=2) as work,
            tc.tile_pool(name="accp", bufs=1) as accp,
        ):
            acc_sb = accp.tile([P, NT], f32)
            fmax = max(TILE_SIZES)
            off = 0
            for j, f in enumerate(TILE_SIZES):
                tp = io_pool.tile([P, fmax], f32, tag="p")
                nc.sync.dma_start(tp[:, :f], p_ap[:, off : off + f])
                tt = io_pool.tile([P, fmax], f32, tag="t")
                nc.sync.dma_start(tt[:, :f], t_ap[:, off : off + f])
                diff = work.tile([P, fmax], f32, tag="diff")
                nc.vector.tensor_sub(diff[:, :f], tp[:, :f], tt[:, :f])
                sq = work.tile([P, fmax], f32, tag="sq")
                nc.scalar.activation(
                    sq[:, :f],
                    diff[:, :f],
                    mybir.ActivationFunctionType.Square,
                    accum_out=acc_sb[:, j : j + 1],
                )
                off += f
            nc.sync.dma_start(acc_ap[:], acc_sb[:])
    nc.compile()
    return nc


def _build_raw():
    """Raw-bacc variant: all 5 tile-pairs resident in SBUF (no buffer
    reuse, no load waits), manual semaphores, no Tile preamble/teardown.
    DVE subtract runs in place over the p-tile; ACT Square writes over the
    t-tile. Saves ~3-4 us of Tile framework overhead."""
    import concourse.bass as bass  # noqa: F401
    from concourse import bacc, mybir

    nc = bacc.Bacc(
        "TRN2", debug=False, target_bir_lowering=False, num_devices=N_CORES
    )
    f32 = mybir.dt.float32
    p_ap = nc.dram_tensor("p", [P, FREE], f32, kind="ExternalInput").ap()
    t_ap = nc.dram_tensor("t", [P, FREE], f32, kind="ExternalInput").ap()
    acc_ap = nc.dram_tensor("acc", [P, NT], f32, kind="ExternalOutput").ap()

    p_sb = [nc.alloc_sbuf_tensor(f"psb{j}", [P, f], f32).ap() for j, f in enumerate(TILE_SIZES)]
    t_sb = [nc.alloc_sbuf_tensor(f"tsb{j}", [P, f], f32).ap() for j, f in enumerate(TILE_SIZES)]
    acc_sb = nc.alloc_sbuf_tensor("accsb", [P, NT], f32).ap()

    pair_sems = [nc.alloc_semaphore(f"pair{j}") for j in range(NT)]
    store_sem = nc.alloc_semaphore("store_sem")
    v_sem = nc.alloc_semaphore("v_sem")
    a_sem = nc.alloc_semaphore("a_sem")

    offs = []
    off = 0
    for f in TILE_SIZES:
        offs.append(off)
        off += f

    with nc.Block() as block:

        @block.sync
        def _(sync):
            for j, f in enumerate(TILE_SIZES):
                o = offs[j]
                sync.dma_start(p_sb[j][:], p_ap[:, o : o + f]).then_inc(
                    pair_sems[j], 16
                )
                sync.dma_start(t_sb[j][:], t_ap[:, o : o + f]).then_inc(
                    pair_sems[j], 16
                )
            # No explicit wait on the store: the Block-exit drains / NRT
            # completion quiesce outstanding HWDGE DMAs, so the exit
            # barrier overlaps the store's flight instead of serializing
            # after it.

        @block.vector
        def _(vector):
            for j in range(NT):
                vector.wait_ge(pair_sems[j], 32)
                vector.tensor_sub(p_sb[j][:], p_sb[j][:], t_sb[j][:]).then_inc(
                    v_sem, 1
                )

        @block.scalar
        def _(scalar):
            for j in range(NT):
                scalar.wait_ge(v_sem, j + 1)
                scalar.activation(
                    t_sb[j][:],
                    p_sb[j][:],
                    mybir.ActivationFunctionType.Square,
                    accum_out=acc_sb[:, j : j + 1],
                ).then_inc(a_sem, 1)
            # Scalar is an HWDGE engine; issuing the store here (right after
            # the last accumulator read) skips a cross-engine sem hop. The
            # self-wait on a_sem makes the last accumulator write visible
            # before the SDMA engines read acc_sb.
            scalar.wait_ge(a_sem, NT)
            scalar.dma_start(acc_ap[:], acc_sb[:]).then_inc(store_sem, 16)

    nc.compile()
    return nc


def _get_nc():
    if "nc" not in _CACHE:
        import os

        if os.environ.get("KERNEL_RAW", "1") == "1":
            _CACHE["nc"] = _build_raw()
        else:
            _CACHE["nc"] = _build()
    return _CACHE["nc"]


def _shard(arr):
    # (B, S, C) contiguous -> 8 contiguous views of [128, FREE]
    return np.ascontiguousarray(arr).reshape(N_CORES, P, FREE)


def _run(in_maps, **kwargs):
    from concourse.bass_utils import run_bass_kernel_spmd

    return run_bass_kernel_spmd(_get_nc(), in_maps, list(range(N_CORES)), **kwargs)


def kernel(predictions, targets, d, batch_size, **_ignored):
    d_i = int(np.asarray(d))
    bs = int(np.asarray(batch_size))
    s_i = 2 * d_i + 1

    pred = np.asarray(predictions, dtype=np.float32)
    targ = np.asarray(targets, dtype=np.float32)

    if bs != B or s_i != S or pred.shape != (B, S, C):
        # Shape fell outside the compiled layout; numpy fallback keeps the
        # contract correct for any input.
        diff = (pred[:bs, :s_i, :C] - targ[:bs, :s_i, :C]).astype(np.float64)
        return np.float32((diff * diff).sum() / s_i / bs)

    pv = _shard(pred)
    tv = _shard(targ)
    in_maps = [{"p": pv[c], "t": tv[c]} for c in range(N_CORES)]
    res = _run(in_maps).results

    total = 0.0
    for r in res:
        total += float(r["acc"].astype(np.float64).sum())
    return np.float32(total / s_i / bs)

